# revision 47
# baseline (speedup 1.0000x reference)
"""Trainium2 Bass kernel for nn_CEBlock (Clifford-algebra equivariant block).

Data-parallel over batch across 8 NeuronCores. Self-contained: hardcodes
shapes (B=32768, Cin=Cout=128, 8 blades) and derives the Cayley table
locally.
"""

import itertools
from contextlib import ExitStack

import numpy as np

# ---------------------------------------------------------------- constants
METRIC = (1, 1, 1)
EPS = 1e-6
BG = np.array([0, 1, 1, 1, 2, 2, 2, 3])  # grade of each blade
C_HALF = float(2.0 ** -0.5)

B_FULL, CIN, CO = 32768, 128, 128
N_CORES = 8
BS = B_FULL // N_CORES          # rows per core
T = 512                         # batch tile (free dim) for compute planes
CHUNK = 128                     # batch chunk (partition dim) for DMA/transpose


def _build_cayley():
    n = len(METRIC)
    blades = [tuple(c) for g in range(n + 1) for c in itertools.combinations(range(n), g)]
    idx = {b: i for i, b in enumerate(blades)}

    def mul(a, b):
        arr = list(a) + list(b)
        sign = 1
        i = 0
        while i < len(arr) - 1:
            if arr[i] > arr[i + 1]:
                arr[i], arr[i + 1] = arr[i + 1], arr[i]
                sign = -sign
                i = max(i - 1, 0)
            elif arr[i] == arr[i + 1]:
                sign *= METRIC[arr[i]]
                del arr[i:i + 2]
                i = max(i - 1, 0)
            else:
                i += 1
        return sign, tuple(arr)

    C = np.zeros((8, 8, 8), np.float32)
    for i, a in enumerate(blades):
        for k, b in enumerate(blades):
            s, r = mul(a, b)
            C[i, idx[r], k] += s
    return C


CAYLEY = _build_cayley()
# j = JTAB[i,k] is the unique output blade of blade_i * blade_k, SIGN its sign
JTAB = np.zeros((8, 8), np.int64)
SIGN = np.zeros((8, 8), np.float32)
for _i in range(8):
    for _k in range(8):
        _nz = np.nonzero(CAYLEY[_i, :, _k])[0]
        assert len(_nz) == 1
        JTAB[_i, _k] = _nz[0]
        SIGN[_i, _k] = CAYLEY[_i, _nz[0], _k]
# ITAB[j,k] = the unique i with blade_i * blade_k = +-blade_j
ITAB = np.zeros((8, 8), np.int64)
for _i in range(8):
    for _k in range(8):
        ITAB[JTAB[_i, _k], _k] = _i
# blade index <-> basis bitmask (blades sorted by grade swap slots 3 and 4)
B2M = np.array([0, 1, 2, 4, 3, 5, 6, 7])
M2B = np.array([0, 1, 2, 4, 3, 5, 6, 7])
PC = np.array([0, 1, 1, 2, 1, 2, 2, 3])  # popcount(mask) = grade


def _sigmoid(z):
    return 1.0 / (1.0 + np.exp(-z))


# ---------------------------------------------------------------- host prep
def _host_constants(W_qkv, b_qkv, silu_a, silu_b, gp_w, norm_a, ln_a):
    """Precompute all per-channel constant tables (tiny, host numpy)."""
    sa = np.asarray(silu_a, np.float32).reshape(CO, 4)
    sb = np.asarray(silu_b, np.float32).reshape(CO, 4)
    gpw = np.asarray(gp_w, np.float32)
    na = np.asarray(norm_a, np.float32)
    lna = np.asarray(ln_a, np.float32)
    W = np.asarray(W_qkv, np.float32)
    b = np.asarray(b_qkv, np.float32)

    # qkv weights, stationary layout [m, j=(proj*4+g), n]; q gets the final
    # 2^-1/2 folded in.
    wqkv = np.zeros((CIN, 12, CO), np.float32)
    for proj in range(3):
        for g in range(4):
            blk = W[proj * CO:(proj + 1) * CO, :, g]      # (n, m)
            scale = C_HALF if proj == 0 else 1.0
            wqkv[:, proj * 4 + g, :] = (blk * scale).T
    wqkv16 = wqkv.astype(np.float16)

    # ln_a (and 2^-1/2) folding into the geometric-product coefficients.
    lnai = lna * C_HALF
    fold_ok = np.abs(lnai).min() > 1e-3
    fold = lnai if fold_ok else np.full(CO, C_HALF, np.float32)

    # diag coefficient tables for the 64 Cayley terms, indexed by output MASK
    # mj and right-factor MASK mk: the left factor has mask mj^mk.
    diagw = np.zeros((CO, 64, CO), np.float32)
    for mj in range(8):
        for mk in range(8):
            i_b = M2B[mj ^ mk]
            k_b = M2B[mk]
            j_b = M2B[mj]
            coef = SIGN[i_b, k_b] * gpw[:, BG[i_b], BG[j_b], BG[k_b]] * fold
            diagw[np.arange(CO), mj * 8 + mk, np.arange(CO)] = coef
    diagw16 = diagw.astype(np.float16)

    # column-sum weights for the channel-mean of |kv| (undo the fold, /CO)
    colw = (1.0 / (CO * np.abs(fold))).astype(np.float16).reshape(CO, 1)

    # the non-folded residual per-channel scale applied to kv-hat (only when
    # folding was unsafe); 1.0 otherwise
    resid = (lnai / fold).astype(np.float32)
    need_resid = not fold_ok

    sna = _sigmoid(na)  # (CO, 4)
    scal = np.zeros((CO, 20), np.float32)
    scal[:, 0] = b[:CO] * C_HALF           # bq'
    scal[:, 1] = b[CO:2 * CO]              # bk
    scal[:, 2] = b[2 * CO:3 * CO]          # bv
    scal[:, 3] = sa[:, 0] / C_HALF         # silu_a g0 (q scaled by c -> /c)
    scal[:, 4:7] = sa[:, 1:4] / (C_HALF ** 2)  # grades 1-3 use q^2 -> /c^2
    scal[:, 7:11] = sb                     # silu_b
    scal[:, 11:15] = sna ** 2              # sna^2 (sqrt scale)
    scal[:, 15:19] = 1.0 - sna + EPS       # den constant
    scal[:, 19] = resid

    onesr = np.ones((1, CO), np.float16)
    return dict(wqkv=wqkv16, diagw=diagw16, scal=scal, colw=colw, onesr=onesr), need_resid


# ---------------------------------------------------------------- bass build
def _build_nc(n_rows, need_resid):
    import concourse.bacc as bacc
    import concourse.bass as bass
    import concourse.tile as tile
    import concourse.masks as masks
    from concourse import mybir

    f32 = mybir.dt.float32
    f16 = mybir.dt.float16
    ALU = mybir.AluOpType
    ACT = mybir.ActivationFunctionType

    nt = n_rows // T
    nchunk_t = T // CHUNK

    nc = bacc.Bacc("TRN2", target_bir_lowering=False, debug=False,
                   num_devices=N_CORES)
    x_ext = nc.declare_dram_parameter("x", [n_rows, CIN, 8], f32, isOutput=False)
    w_ext = nc.declare_dram_parameter("wqkv", [CIN, 12, CO], f16, isOutput=False)
    d_ext = nc.declare_dram_parameter("diagw", [CO, 64, CO], f16, isOutput=False)
    s_ext = nc.declare_dram_parameter("scal", [CO, 20], f32, isOutput=False)
    cw_ext = nc.declare_dram_parameter("colw", [CO, 1], f16, isOutput=False)
    or_ext = nc.declare_dram_parameter("onesr", [1, CO], f16, isOutput=False)
    out_ext = nc.declare_dram_parameter("out", [n_rows, CIN, 8], f32, isOutput=True)

    with tile.TileContext(nc) as tc, ExitStack() as ctx:
        consts = ctx.enter_context(tc.tile_pool(name="consts", bufs=1))
        pin = ctx.enter_context(tc.tile_pool(name="pin", bufs=2))
        pbig = ctx.enter_context(tc.tile_pool(name="pbig", bufs=2))
        psmall = ctx.enter_context(tc.tile_pool(name="psmall", bufs=1))
        pprod = ctx.enter_context(tc.tile_pool(name="pprod", bufs=2))
        pout = ctx.enter_context(tc.tile_pool(name="pout", bufs=2))
        pxT = ctx.enter_context(tc.tile_pool(name="pxT", bufs=2))
        psum_mm = ctx.enter_context(tc.tile_pool(name="psum_mm", bufs=2, space="PSUM"))
        psum_kv = ctx.enter_context(tc.tile_pool(name="psum_kv", bufs=2, space="PSUM"))
        psum_x = ctx.enter_context(tc.tile_pool(name="psum_x", bufs=1, space="PSUM"))
        psum_o = ctx.enter_context(tc.tile_pool(name="psum_o", bufs=1, space="PSUM"))

        w_sb = consts.tile([CIN, 12, CO], f16)
        nc.scalar.dma_start(out=w_sb, in_=w_ext[:])
        d_sb = consts.tile([CO, 64, CO], f16)
        nc.scalar.dma_start(out=d_sb, in_=d_ext[:])
        s_sb = consts.tile([CO, 20], f32)
        nc.scalar.dma_start(out=s_sb, in_=s_ext[:])
        cw_sb = consts.tile([CO, 1], f16)
        nc.scalar.dma_start(out=cw_sb, in_=cw_ext[:])
        or_sb = consts.tile([1, CO], f16)
        nc.scalar.dma_start(out=or_sb, in_=or_ext[:])
        ident16 = consts.tile([128, 128], f16)
        masks.make_identity(nc, ident16[:])

        groups = [(0,), (1, 2), (3, 4), (5, 6), (7,)]

        def phase_AB(t):
            h = {}
            xT = pxT.tile([CIN, 8, T], f16, tag="xT")
            for c in range(nchunk_t):
                r0 = t * T + c * CHUNK
                xf = pin.tile([CHUNK, CIN, 8], f32, tag="xf")
                nc.sync.dma_start(out=xf, in_=x_ext[r0:r0 + CHUNK])
                xb = pin.tile([CHUNK, 8, CIN], f16, tag="xb")
                nc.gpsimd.tensor_copy(xb[:], xf[:].rearrange("p m i -> p i m"))
                xtp = psum_x.tile([CIN, 8, CHUNK], f16, tag="xtp")
                for i in range(8):
                    nc.tensor.transpose(xtp[:, i, :], xb[:, i, :], ident16[:])
                nc.scalar.activation(out=xT[:, :, c * CHUNK:(c + 1) * CHUNK],
                                     in_=xtp, func=ACT.Copy)
            qb = pbig.tile([CO, 8, T], f16, tag="qb")
            kb = pbig.tile([CO, 8, T], f16, tag="kb")
            vb = pbig.tile([CO, 8, T], f16, tag="vb")
            dests = [qb, kb, vb]
            for proj in range(3):
                for grp in groups:
                    ps = psum_mm.tile([CO, 2, T], f32, tag="mm")
                    for n, i in enumerate(grp):
                        nc.tensor.matmul(ps[:, n, :],
                                         lhsT=w_sb[:, proj * 4 + BG[i], :],
                                         rhs=xT[:, i, :], start=True, stop=True)
                    slots = [int(B2M[i]) if proj == 1 else i for i in grp]
                    if grp == (0,):
                        nc.scalar.activation(
                            out=dests[proj][:, 0, :], in_=ps[:, 0, :],
                            func=ACT.Identity, bias=s_sb[:, proj:proj + 1],
                            scale=1.0)
                    elif len(grp) == 1:
                        nc.scalar.activation(out=dests[proj][:, slots[0], :],
                                             in_=ps[:, 0, :], func=ACT.Copy)
                    else:
                        s0, s1 = slots
                        dst = dests[proj][:]
                        dst_ap = bass.AP(tensor=dst.tensor,
                                         offset=dst.offset + s0 * T,
                                         ap=[dst.ap[0], [(s1 - s0) * T, 2], [1, T]])
                        nc.scalar.activation(out=dst_ap, in_=ps[:, 0:2, :],
                                             func=ACT.Copy)
            h.update(xT=xT, qb=qb, kb=kb, vb=vb)
            return h

        def phase_C12(h):
            qb, vb = h["qb"], h["vb"]
            sqq = psmall.tile([CO, 7, T], f16, tag="sqshared")
            nc.vector.tensor_tensor(sqq[:], qb[:, 1:8, :], qb[:, 1:8, :], ALU.mult)
            s_q = psmall.tile([CO, 2, T], f16, tag="s_q")
            nc.vector.tensor_tensor(s_q[:], sqq[:, 0:4:3, :], sqq[:, 1:5:3, :], ALU.add)
            nc.vector.tensor_tensor(s_q[:], s_q[:], sqq[:, 2:6:3, :], ALU.add)
            sig = psmall.tile([CO, 4, T], f16, tag="sig")
            nc.scalar.activation(out=sig[:, 0, :], in_=qb[:, 0, :], func=ACT.Sigmoid,
                                 scale=s_sb[:, 3:4], bias=s_sb[:, 7:8])
            nc.scalar.activation(out=sig[:, 1, :], in_=s_q[:, 0, :], func=ACT.Sigmoid,
                                 scale=s_sb[:, 4:5], bias=s_sb[:, 8:9])
            nc.scalar.activation(out=sig[:, 2, :], in_=s_q[:, 1, :], func=ACT.Sigmoid,
                                 scale=s_sb[:, 5:6], bias=s_sb[:, 9:10])
            nc.scalar.activation(out=sig[:, 3, :], in_=sqq[:, 6, :], func=ACT.Sigmoid,
                                 scale=s_sb[:, 6:7], bias=s_sb[:, 10:11])
            tmpq = psmall.tile([CO, T], f16, tag="tmpq")
            nc.vector.tensor_tensor(tmpq[:], qb[:, 4, :], sig[:, 2, :], ALU.mult)
            nc.vector.tensor_tensor(qb[:, 4, :], qb[:, 3, :], sig[:, 1, :], ALU.mult)
            nc.vector.tensor_copy(qb[:, 3, :], tmpq[:])
            for i in (0, 1, 2, 5, 6, 7):
                nc.vector.tensor_tensor(qb[:, i, :], qb[:, i, :],
                                        sig[:, BG[i], :], ALU.mult)
            sqv = psmall.tile([CO, 8, T], f16, tag="svshared")
            nc.vector.tensor_tensor(sqv[:], vb[:], vb[:], ALU.mult)
            s_v = psmall.tile([CO, 2, T], f16, tag="s_v")
            nc.vector.tensor_tensor(s_v[:], sqv[:, 1:5:3, :], sqv[:, 2:6:3, :], ALU.add)
            nc.vector.tensor_tensor(s_v[:], s_v[:], sqv[:, 3:7:3, :], ALU.add)
            den = psmall.tile([CO, 4, T], f32, tag="den")
            srcs = [sqv[:, 0, :], s_v[:, 0, :], s_v[:, 1, :], sqv[:, 7, :]]
            for g in range(4):
                nc.scalar.activation(out=den[:, g, :], in_=srcs[g], func=ACT.Sqrt,
                                     scale=s_sb[:, 11 + g:12 + g])
                nc.vector.tensor_scalar(den[:, g, :], den[:, g, :],
                                        s_sb[:, 15 + g:16 + g], None, ALU.add)
            r4 = psmall.tile([CO, 4, T], f32, tag="r4")
            for g in range(4):
                nc.vector.reciprocal_approx_fast(out=r4[:, g, :], in_=den[:, g, :])
            r16 = psmall.tile([CO, 4, T], f16, tag="r16")
            nc.vector.tensor_copy(r16[:], r4[:])
            tmpv = psmall.tile([CO, T], f16, tag="tmpv")
            nc.vector.tensor_tensor(tmpv[:], vb[:, 4, :], r16[:, 2, :], ALU.mult)
            nc.vector.tensor_tensor(vb[:, 4, :], vb[:, 3, :], r16[:, 1, :], ALU.mult)
            nc.vector.tensor_copy(vb[:, 3, :], tmpv[:])
            for i in (0, 1, 2, 5, 6, 7):
                nc.vector.tensor_tensor(vb[:, i, :], vb[:, i, :],
                                        r16[:, BG[i], :], ALU.mult)
            h["kvs"] = pbig.tile([CO, 8, T], f16, name="kvs", tag="kvs")
            h["sqkv"] = psmall.tile([CO, 8, T], f16, name="sqkv", tag="sqshared")

        def phase_C3(h, mjs):
            kb, vt, kvs = h["kb"], h["vb"], h["kvs"]
            kb_ap = kb[:]
            vt_ap = vt[:]
            pstride = kb_ap.ap[0][0]
            for mj in mjs:
                b2, b1, b0 = (mj >> 2) & 1, (mj >> 1) & 1, mj & 1
                s2, s1, s0 = 1 - 2 * b2, 1 - 2 * b1, 1 - 2 * b0
                p = pprod.tile([CO, 2, 2, 2, T], f16, tag="p")
                p_ap = p[:]
                base = kb_ap.offset + mj * T
                # walrus allows at most 3 free dims; merge a ratio-2
                # same-sign XOR axis pair (exists unless mj in {2, 5})
                if b2 == b1:
                    in0 = bass.AP(tensor=kb_ap.tensor, offset=base,
                                  ap=[[pstride, CO], [s2 * 2 * T, 4],
                                      [s0 * T, 2], [1, T]])
                    in1 = bass.AP(tensor=vt_ap.tensor, offset=vt_ap.offset,
                                  ap=[[pstride, CO], [2 * T, 4], [T, 2], [1, T]])
                    out = bass.AP(tensor=p_ap.tensor, offset=p_ap.offset,
                                  ap=[[p_ap.ap[0][0], CO], [2 * T, 4], [T, 2], [1, T]])
                    nc.vector.tensor_tensor(out, in0, in1, ALU.mult)
                elif b1 == b0:
                    in0 = bass.AP(tensor=kb_ap.tensor, offset=base,
                                  ap=[[pstride, CO], [s2 * 4 * T, 2],
                                      [s1 * T, 4], [1, T]])
                    in1 = bass.AP(tensor=vt_ap.tensor, offset=vt_ap.offset,
                                  ap=[[pstride, CO], [4 * T, 2], [T, 4], [1, T]])
                    out = bass.AP(tensor=p_ap.tensor, offset=p_ap.offset,
                                  ap=[[p_ap.ap[0][0], CO], [4 * T, 2], [T, 4], [1, T]])
                    nc.vector.tensor_tensor(out, in0, in1, ALU.mult)
                else:
                    for a in (0, 1):
                        sub = (mj ^ (a << 2)) * T
                        in0 = bass.AP(tensor=kb_ap.tensor,
                                      offset=kb_ap.offset + sub,
                                      ap=[[pstride, CO], [s1 * 2 * T, 2],
                                          [s0 * T, 2], [1, T]])
                        in1 = bass.AP(tensor=vt_ap.tensor,
                                      offset=vt_ap.offset + a * 4 * T,
                                      ap=[[pstride, CO], [2 * T, 2], [T, 2], [1, T]])
                        out = bass.AP(tensor=p_ap.tensor,
                                      offset=p_ap.offset + a * 4 * T,
                                      ap=[[p_ap.ap[0][0], CO], [2 * T, 2],
                                          [T, 2], [1, T]])
                        nc.vector.tensor_tensor(out, in0, in1, ALU.mult)
                kvp = psum_kv.tile([CO, T], f32, tag="kv")
                for mk in range(8):
                    nc.tensor.matmul(kvp, lhsT=d_sb[:, mj * 8 + mk, :],
                                     rhs=p[:, (mk >> 2) & 1, (mk >> 1) & 1, mk & 1, :],
                                     start=(mk == 0), stop=(mk == 7))
                nc.scalar.activation(out=kvs[:, mj, :], in_=kvp, func=ACT.Copy)
                nc.vector.tensor_tensor(h["sqkv"][:, mj, :], kvs[:, mj, :],
                                        kvs[:, mj, :], ALU.mult)

        def phase_C4a(h):
            kvs = h["kvs"]
            sqkv = h["sqkv"]
            t4 = psmall.tile([CO, 4, T], f16, tag="svshared")
            nc.vector.tensor_tensor(t4[:], sqkv[:, 0:4, :], sqkv[:, 4:8, :], ALU.add)
            nc.vector.tensor_tensor(t4[:, 0:2, :], t4[:, 0:2, :], t4[:, 2:4, :], ALU.add)
            n2 = psmall.tile([CO, T], f32, tag="n2")
            nc.vector.tensor_tensor(n2[:], t4[:, 0, :], t4[:, 1, :], ALU.add)
            sroot = psmall.tile([CO, T], f16, tag="sroot")
            nc.scalar.activation(out=sroot, in_=n2, func=ACT.Sqrt)
            csp = psum_kv.tile([1, T], f32, tag="kv")
            nc.tensor.matmul(csp, lhsT=cw_sb, rhs=sroot, start=True, stop=True)
            rb0 = psmall.tile([1, T], f32, tag="rb0")
            nc.vector.tensor_scalar(rb0[:], csp[:], EPS, None, ALU.add)
            rb1 = psmall.tile([1, T], f32, tag="rb1")
            nc.vector.reciprocal_approx_fast(out=rb1, in_=rb0)
            rb1h = psmall.tile([1, T], f16, tag="rb1h")
            nc.vector.tensor_copy(rb1h[:], rb1[:])
            rbp = psum_kv.tile([CO, T], f32, tag="kv")
            nc.tensor.matmul(rbp, lhsT=or_sb, rhs=rb1h, start=True, stop=True)
            rbB = psmall.tile([CO, T], f16, tag="rbB")
            nc.scalar.activation(out=rbB, in_=rbp, func=ACT.Copy)
            h["rbB"] = rbB

        def phase_C4b(h):
            kvs, qt, rbB = h["kvs"], h["qb"], h["rbB"]
            outp = pout.tile([CO, 8, T], f16, tag="outp")
            rbB_b = rbB[:, None, :].to_broadcast((CO, 8, T))
            nc.vector.tensor_tensor(kvs[:], kvs[:], rbB_b, ALU.mult)
            if need_resid:
                nc.vector.tensor_scalar(kvs[:], kvs[:], s_sb[:, 19:20], None, ALU.mult)
            nc.vector.tensor_tensor(outp[:], kvs[:], qt[:], ALU.add)
            h["outp"] = outp

        def phase_D(t, h):
            outp = h["outp"]
            for c in range(nchunk_t):
                r0 = t * T + c * CHUNK
                otp = psum_o.tile([CHUNK, 8, CIN], f16, tag="otp")
                for i in range(8):
                    nc.tensor.transpose(otp[:, i, :],
                                        outp[:, B2M[i], c * CHUNK:(c + 1) * CHUNK],
                                        ident16[:])
                st32 = pout.tile([CHUNK, CIN, 8], f32, tag="st32")
                nc.scalar.activation(out=st32[:].rearrange("p m i -> p i m"),
                                     in_=otp, func=ACT.Copy)
                nc.scalar.dma_start(out=out_ext[r0:r0 + CHUNK], in_=st32)

        for t in range(nt):
            h = phase_AB(t)
            phase_C12(h)
            phase_C3(h, range(8))
            phase_C4a(h)
            phase_C4b(h)
            phase_D(t, h)

    return nc


_NC_CACHE = {}


def _get_nc(n_rows, need_resid):
    key = (n_rows, need_resid)
    if key not in _NC_CACHE:
        _NC_CACHE[key] = _build_nc(n_rows, need_resid)
    return _NC_CACHE[key]


# ---------------------------------------------------------------- entrypoint
def kernel(input, W_qkv, b_qkv, silu_a, silu_b, gp_w, norm_a, ln_a):
    from concourse.bass_utils import run_bass_kernel_spmd

    x = np.ascontiguousarray(np.asarray(input, np.float32))
    consts, need_resid = _host_constants(W_qkv, b_qkv, silu_a, silu_b,
                                         gp_w, norm_a, ln_a)
    nc = _get_nc(BS, need_resid)

    in_maps = []
    for c in range(N_CORES):
        m = {"x": x[c * BS:(c + 1) * BS]}
        m.update(consts)
        in_maps.append(m)

    if not nc.is_finalized():
        nc.finalize()
    res = run_bass_kernel_spmd(nc, in_maps, core_ids=list(range(N_CORES)))
    outs = [res.results[c]["out"].reshape(BS, CIN, 8) for c in range(N_CORES)]
    return np.concatenate(outs, axis=0).astype(np.float32)


# revision 48
# speedup vs baseline: 1.0042x; 1.0042x over previous
"""Trainium2 Bass kernel for nn_CEBlock (Clifford-algebra equivariant block).

Data-parallel over batch across 8 NeuronCores. Self-contained: hardcodes
shapes (B=32768, Cin=Cout=128, 8 blades) and derives the Cayley table
locally.
"""

import itertools
from contextlib import ExitStack

import numpy as np

# ---------------------------------------------------------------- constants
METRIC = (1, 1, 1)
EPS = 1e-6
BG = np.array([0, 1, 1, 1, 2, 2, 2, 3])  # grade of each blade
C_HALF = float(2.0 ** -0.5)

B_FULL, CIN, CO = 32768, 128, 128
N_CORES = 8
BS = B_FULL // N_CORES          # rows per core
T = 512                         # batch tile (free dim) for compute planes
CHUNK = 128                     # batch chunk (partition dim) for DMA/transpose


def _build_cayley():
    n = len(METRIC)
    blades = [tuple(c) for g in range(n + 1) for c in itertools.combinations(range(n), g)]
    idx = {b: i for i, b in enumerate(blades)}

    def mul(a, b):
        arr = list(a) + list(b)
        sign = 1
        i = 0
        while i < len(arr) - 1:
            if arr[i] > arr[i + 1]:
                arr[i], arr[i + 1] = arr[i + 1], arr[i]
                sign = -sign
                i = max(i - 1, 0)
            elif arr[i] == arr[i + 1]:
                sign *= METRIC[arr[i]]
                del arr[i:i + 2]
                i = max(i - 1, 0)
            else:
                i += 1
        return sign, tuple(arr)

    C = np.zeros((8, 8, 8), np.float32)
    for i, a in enumerate(blades):
        for k, b in enumerate(blades):
            s, r = mul(a, b)
            C[i, idx[r], k] += s
    return C


CAYLEY = _build_cayley()
# j = JTAB[i,k] is the unique output blade of blade_i * blade_k, SIGN its sign
JTAB = np.zeros((8, 8), np.int64)
SIGN = np.zeros((8, 8), np.float32)
for _i in range(8):
    for _k in range(8):
        _nz = np.nonzero(CAYLEY[_i, :, _k])[0]
        assert len(_nz) == 1
        JTAB[_i, _k] = _nz[0]
        SIGN[_i, _k] = CAYLEY[_i, _nz[0], _k]
# ITAB[j,k] = the unique i with blade_i * blade_k = +-blade_j
ITAB = np.zeros((8, 8), np.int64)
for _i in range(8):
    for _k in range(8):
        ITAB[JTAB[_i, _k], _k] = _i
# blade index <-> basis bitmask (blades sorted by grade swap slots 3 and 4)
B2M = np.array([0, 1, 2, 4, 3, 5, 6, 7])
M2B = np.array([0, 1, 2, 4, 3, 5, 6, 7])
PC = np.array([0, 1, 1, 2, 1, 2, 2, 3])  # popcount(mask) = grade


def _sigmoid(z):
    return 1.0 / (1.0 + np.exp(-z))


# ---------------------------------------------------------------- host prep
def _host_constants(W_qkv, b_qkv, silu_a, silu_b, gp_w, norm_a, ln_a):
    """Precompute all per-channel constant tables (tiny, host numpy)."""
    sa = np.asarray(silu_a, np.float32).reshape(CO, 4)
    sb = np.asarray(silu_b, np.float32).reshape(CO, 4)
    gpw = np.asarray(gp_w, np.float32)
    na = np.asarray(norm_a, np.float32)
    lna = np.asarray(ln_a, np.float32)
    W = np.asarray(W_qkv, np.float32)
    b = np.asarray(b_qkv, np.float32)

    # qkv weights, stationary layout [m, j=(proj*4+g), n]; q gets the final
    # 2^-1/2 folded in.
    wqkv = np.zeros((CIN, 12, CO), np.float32)
    for proj in range(3):
        for g in range(4):
            blk = W[proj * CO:(proj + 1) * CO, :, g]      # (n, m)
            scale = C_HALF if proj == 0 else 1.0
            wqkv[:, proj * 4 + g, :] = (blk * scale).T
    wqkv16 = wqkv.astype(np.float16)

    # ln_a (and 2^-1/2) folding into the geometric-product coefficients.
    lnai = lna * C_HALF
    fold_ok = np.abs(lnai).min() > 1e-3
    fold = lnai if fold_ok else np.full(CO, C_HALF, np.float32)

    # diag coefficient tables for the 64 Cayley terms, indexed by output MASK
    # mj and right-factor MASK mk: the left factor has mask mj^mk.
    diagw = np.zeros((CO, 64, CO), np.float32)
    for mj in range(8):
        for mk in range(8):
            i_b = M2B[mj ^ mk]
            k_b = M2B[mk]
            j_b = M2B[mj]
            coef = SIGN[i_b, k_b] * gpw[:, BG[i_b], BG[j_b], BG[k_b]] * fold
            diagw[np.arange(CO), mj * 8 + mk, np.arange(CO)] = coef
    diagw16 = diagw.astype(np.float16)

    # column-sum weights for the channel-mean of |kv| (undo the fold, /CO)
    colw = (1.0 / (CO * np.abs(fold))).astype(np.float16).reshape(CO, 1)

    # the non-folded residual per-channel scale applied to kv-hat (only when
    # folding was unsafe); 1.0 otherwise
    resid = (lnai / fold).astype(np.float32)
    need_resid = not fold_ok

    sna = _sigmoid(na)  # (CO, 4)
    scal = np.zeros((CO, 20), np.float32)
    scal[:, 0] = b[:CO] * C_HALF           # bq'
    scal[:, 1] = b[CO:2 * CO]              # bk
    scal[:, 2] = b[2 * CO:3 * CO]          # bv
    scal[:, 3] = sa[:, 0] / C_HALF         # silu_a g0 (q scaled by c -> /c)
    scal[:, 4:7] = sa[:, 1:4] / (C_HALF ** 2)  # grades 1-3 use q^2 -> /c^2
    scal[:, 7:11] = sb                     # silu_b
    scal[:, 11:15] = sna ** 2              # sna^2 (sqrt scale)
    scal[:, 15:19] = 1.0 - sna + EPS       # den constant
    scal[:, 19] = resid

    onesr = np.ones((1, CO), np.float16)
    return dict(wqkv=wqkv16, diagw=diagw16, scal=scal, colw=colw, onesr=onesr), need_resid


# ---------------------------------------------------------------- bass build
def _build_nc(n_rows, need_resid):
    import concourse.bacc as bacc
    import concourse.bass as bass
    import concourse.tile as tile
    import concourse.masks as masks
    from concourse import mybir

    f32 = mybir.dt.float32
    f16 = mybir.dt.float16
    ALU = mybir.AluOpType
    ACT = mybir.ActivationFunctionType

    nt = n_rows // T
    nchunk_t = T // CHUNK

    nc = bacc.Bacc("TRN2", target_bir_lowering=False, debug=False,
                   num_devices=N_CORES)
    x_ext = nc.declare_dram_parameter("x", [n_rows, CIN, 8], f32, isOutput=False)
    w_ext = nc.declare_dram_parameter("wqkv", [CIN, 12, CO], f16, isOutput=False)
    d_ext = nc.declare_dram_parameter("diagw", [CO, 64, CO], f16, isOutput=False)
    s_ext = nc.declare_dram_parameter("scal", [CO, 20], f32, isOutput=False)
    cw_ext = nc.declare_dram_parameter("colw", [CO, 1], f16, isOutput=False)
    or_ext = nc.declare_dram_parameter("onesr", [1, CO], f16, isOutput=False)
    out_ext = nc.declare_dram_parameter("out", [n_rows, CIN, 8], f32, isOutput=True)

    with tile.TileContext(nc) as tc, ExitStack() as ctx:
        consts = ctx.enter_context(tc.tile_pool(name="consts", bufs=1))
        pin = ctx.enter_context(tc.tile_pool(name="pin", bufs=2))
        pbig = ctx.enter_context(tc.tile_pool(name="pbig", bufs=2))
        psmall = ctx.enter_context(tc.tile_pool(name="psmall", bufs=1))
        pprod = ctx.enter_context(tc.tile_pool(name="pprod", bufs=2))
        pout = ctx.enter_context(tc.tile_pool(name="pout", bufs=2))
        pxT = ctx.enter_context(tc.tile_pool(name="pxT", bufs=2))
        psum_mm = ctx.enter_context(tc.tile_pool(name="psum_mm", bufs=2, space="PSUM"))
        psum_kv = ctx.enter_context(tc.tile_pool(name="psum_kv", bufs=2, space="PSUM"))
        psum_x = ctx.enter_context(tc.tile_pool(name="psum_x", bufs=1, space="PSUM"))
        psum_o = ctx.enter_context(tc.tile_pool(name="psum_o", bufs=1, space="PSUM"))

        w_sb = consts.tile([CIN, 12, CO], f16)
        nc.scalar.dma_start(out=w_sb, in_=w_ext[:])
        d_sb = consts.tile([CO, 64, CO], f16)
        nc.scalar.dma_start(out=d_sb, in_=d_ext[:])
        s_sb = consts.tile([CO, 20], f32)
        nc.scalar.dma_start(out=s_sb, in_=s_ext[:])
        cw_sb = consts.tile([CO, 1], f16)
        nc.scalar.dma_start(out=cw_sb, in_=cw_ext[:])
        or_sb = consts.tile([1, CO], f16)
        nc.scalar.dma_start(out=or_sb, in_=or_ext[:])
        ident16 = consts.tile([128, 128], f16)
        masks.make_identity(nc, ident16[:])

        groups = [(0,), (1, 2), (3, 4), (5, 6), (7,)]

        def phase_AB(t):
            h = {}
            xT = pxT.tile([CIN, 8, T], f16, tag="xT")
            for c in range(nchunk_t):
                r0 = t * T + c * CHUNK
                xf = pin.tile([CHUNK, CIN, 8], f32, tag="xf")
                nc.sync.dma_start(out=xf, in_=x_ext[r0:r0 + CHUNK])
                xb = pin.tile([CHUNK, 8, CIN], f16, tag="xb")
                nc.gpsimd.tensor_copy(xb[:], xf[:].rearrange("p m i -> p i m"))
                xtp = psum_x.tile([CIN, 8, CHUNK], f16, tag="xtp")
                for i in range(8):
                    nc.tensor.transpose(xtp[:, i, :], xb[:, i, :], ident16[:])
                nc.scalar.activation(out=xT[:, :, c * CHUNK:(c + 1) * CHUNK],
                                     in_=xtp, func=ACT.Copy)
            qb = pbig.tile([CO, 8, T], f16, tag="qb")
            kb = pbig.tile([CO, 8, T], f16, tag="kb")
            vb = pbig.tile([CO, 8, T], f16, tag="vb")
            dests = [qb, kb, vb]
            for proj in range(3):
                for grp in groups:
                    ps = psum_mm.tile([CO, 2, T], f32, tag="mm")
                    for n, i in enumerate(grp):
                        nc.tensor.matmul(ps[:, n, :],
                                         lhsT=w_sb[:, proj * 4 + BG[i], :],
                                         rhs=xT[:, i, :], start=True, stop=True)
                    slots = [int(B2M[i]) if proj == 1 else i for i in grp]
                    if grp == (0,):
                        nc.scalar.activation(
                            out=dests[proj][:, 0, :], in_=ps[:, 0, :],
                            func=ACT.Identity, bias=s_sb[:, proj:proj + 1],
                            scale=1.0)
                    elif len(grp) == 1:
                        nc.scalar.activation(out=dests[proj][:, slots[0], :],
                                             in_=ps[:, 0, :], func=ACT.Copy)
                    else:
                        s0, s1 = slots
                        dst = dests[proj][:]
                        dst_ap = bass.AP(tensor=dst.tensor,
                                         offset=dst.offset + s0 * T,
                                         ap=[dst.ap[0], [(s1 - s0) * T, 2], [1, T]])
                        nc.scalar.activation(out=dst_ap, in_=ps[:, 0:2, :],
                                             func=ACT.Copy)
            h.update(xT=xT, qb=qb, kb=kb, vb=vb)
            return h

        def phase_C12(h):
            qb, vb = h["qb"], h["vb"]
            sqq = psmall.tile([CO, 7, T], f16, tag="sqshared")
            nc.vector.tensor_tensor(sqq[:], qb[:, 1:8, :], qb[:, 1:8, :], ALU.mult)
            s_q = psmall.tile([CO, 2, T], f16, tag="s_q")
            nc.vector.tensor_tensor(s_q[:], sqq[:, 0:4:3, :], sqq[:, 1:5:3, :], ALU.add)
            nc.vector.tensor_tensor(s_q[:], s_q[:], sqq[:, 2:6:3, :], ALU.add)
            sig = psmall.tile([CO, 4, T], f16, tag="sig")
            nc.scalar.activation(out=sig[:, 0, :], in_=qb[:, 0, :], func=ACT.Sigmoid,
                                 scale=s_sb[:, 3:4], bias=s_sb[:, 7:8])
            nc.scalar.activation(out=sig[:, 1, :], in_=s_q[:, 0, :], func=ACT.Sigmoid,
                                 scale=s_sb[:, 4:5], bias=s_sb[:, 8:9])
            nc.scalar.activation(out=sig[:, 2, :], in_=s_q[:, 1, :], func=ACT.Sigmoid,
                                 scale=s_sb[:, 5:6], bias=s_sb[:, 9:10])
            nc.scalar.activation(out=sig[:, 3, :], in_=sqq[:, 6, :], func=ACT.Sigmoid,
                                 scale=s_sb[:, 6:7], bias=s_sb[:, 10:11])
            tmpq = psmall.tile([CO, T], f16, tag="tmpq")
            nc.vector.tensor_tensor(tmpq[:], qb[:, 4, :], sig[:, 2, :], ALU.mult)
            nc.vector.tensor_tensor(qb[:, 4, :], qb[:, 3, :], sig[:, 1, :], ALU.mult)
            nc.vector.tensor_copy(qb[:, 3, :], tmpq[:])
            nc.vector.tensor_tensor(qb[:, 0, :], qb[:, 0, :], sig[:, 0, :], ALU.mult)
            nc.vector.tensor_tensor(qb[:, 1:3, :], qb[:, 1:3, :],
                                    sig[:, 1, None, :].to_broadcast((CO, 2, T)),
                                    ALU.mult)
            nc.vector.tensor_tensor(qb[:, 5:7, :], qb[:, 5:7, :],
                                    sig[:, 2, None, :].to_broadcast((CO, 2, T)),
                                    ALU.mult)
            nc.vector.tensor_tensor(qb[:, 7, :], qb[:, 7, :], sig[:, 3, :], ALU.mult)
            sqv = psmall.tile([CO, 8, T], f16, tag="svshared")
            nc.vector.tensor_tensor(sqv[:], vb[:], vb[:], ALU.mult)
            s_v = psmall.tile([CO, 2, T], f16, tag="s_v")
            nc.vector.tensor_tensor(s_v[:], sqv[:, 1:5:3, :], sqv[:, 2:6:3, :], ALU.add)
            nc.vector.tensor_tensor(s_v[:], s_v[:], sqv[:, 3:7:3, :], ALU.add)
            den = psmall.tile([CO, 4, T], f32, tag="den")
            srcs = [sqv[:, 0, :], s_v[:, 0, :], s_v[:, 1, :], sqv[:, 7, :]]
            for g in range(4):
                nc.scalar.activation(out=den[:, g, :], in_=srcs[g], func=ACT.Sqrt,
                                     scale=s_sb[:, 11 + g:12 + g])
                nc.vector.tensor_scalar(den[:, g, :], den[:, g, :],
                                        s_sb[:, 15 + g:16 + g], None, ALU.add)
            r4 = psmall.tile([CO, 4, T], f32, tag="r4")
            for g in range(4):
                nc.vector.reciprocal_approx_fast(out=r4[:, g, :], in_=den[:, g, :])
            r16 = psmall.tile([CO, 4, T], f16, tag="r16")
            nc.vector.tensor_copy(r16[:], r4[:])
            tmpv = psmall.tile([CO, T], f16, tag="tmpv")
            nc.vector.tensor_tensor(tmpv[:], vb[:, 4, :], r16[:, 2, :], ALU.mult)
            nc.vector.tensor_tensor(vb[:, 4, :], vb[:, 3, :], r16[:, 1, :], ALU.mult)
            nc.vector.tensor_copy(vb[:, 3, :], tmpv[:])
            nc.vector.tensor_tensor(vb[:, 0, :], vb[:, 0, :], r16[:, 0, :], ALU.mult)
            nc.vector.tensor_tensor(vb[:, 1:3, :], vb[:, 1:3, :],
                                    r16[:, 1, None, :].to_broadcast((CO, 2, T)),
                                    ALU.mult)
            nc.vector.tensor_tensor(vb[:, 5:7, :], vb[:, 5:7, :],
                                    r16[:, 2, None, :].to_broadcast((CO, 2, T)),
                                    ALU.mult)
            nc.vector.tensor_tensor(vb[:, 7, :], vb[:, 7, :], r16[:, 3, :], ALU.mult)
            h["kvs"] = pbig.tile([CO, 8, T], f16, name="kvs", tag="kvs")
            h["sqkv"] = psmall.tile([CO, 8, T], f16, name="sqkv", tag="sqshared")

        def phase_C3(h, mjs):
            kb, vt, kvs = h["kb"], h["vb"], h["kvs"]
            kb_ap = kb[:]
            vt_ap = vt[:]
            pstride = kb_ap.ap[0][0]
            for mj in mjs:
                b2, b1, b0 = (mj >> 2) & 1, (mj >> 1) & 1, mj & 1
                s2, s1, s0 = 1 - 2 * b2, 1 - 2 * b1, 1 - 2 * b0
                p = pprod.tile([CO, 2, 2, 2, T], f16, tag="p")
                p_ap = p[:]
                base = kb_ap.offset + mj * T
                # walrus allows at most 3 free dims; merge a ratio-2
                # same-sign XOR axis pair (exists unless mj in {2, 5})
                if b2 == b1:
                    in0 = bass.AP(tensor=kb_ap.tensor, offset=base,
                                  ap=[[pstride, CO], [s2 * 2 * T, 4],
                                      [s0 * T, 2], [1, T]])
                    in1 = bass.AP(tensor=vt_ap.tensor, offset=vt_ap.offset,
                                  ap=[[pstride, CO], [2 * T, 4], [T, 2], [1, T]])
                    out = bass.AP(tensor=p_ap.tensor, offset=p_ap.offset,
                                  ap=[[p_ap.ap[0][0], CO], [2 * T, 4], [T, 2], [1, T]])
                    nc.vector.tensor_tensor(out, in0, in1, ALU.mult)
                elif b1 == b0:
                    in0 = bass.AP(tensor=kb_ap.tensor, offset=base,
                                  ap=[[pstride, CO], [s2 * 4 * T, 2],
                                      [s1 * T, 4], [1, T]])
                    in1 = bass.AP(tensor=vt_ap.tensor, offset=vt_ap.offset,
                                  ap=[[pstride, CO], [4 * T, 2], [T, 4], [1, T]])
                    out = bass.AP(tensor=p_ap.tensor, offset=p_ap.offset,
                                  ap=[[p_ap.ap[0][0], CO], [4 * T, 2], [T, 4], [1, T]])
                    nc.vector.tensor_tensor(out, in0, in1, ALU.mult)
                else:
                    for a in (0, 1):
                        sub = (mj ^ (a << 2)) * T
                        in0 = bass.AP(tensor=kb_ap.tensor,
                                      offset=kb_ap.offset + sub,
                                      ap=[[pstride, CO], [s1 * 2 * T, 2],
                                          [s0 * T, 2], [1, T]])
                        in1 = bass.AP(tensor=vt_ap.tensor,
                                      offset=vt_ap.offset + a * 4 * T,
                                      ap=[[pstride, CO], [2 * T, 2], [T, 2], [1, T]])
                        out = bass.AP(tensor=p_ap.tensor,
                                      offset=p_ap.offset + a * 4 * T,
                                      ap=[[p_ap.ap[0][0], CO], [2 * T, 2],
                                          [T, 2], [1, T]])
                        nc.vector.tensor_tensor(out, in0, in1, ALU.mult)
                kvp = psum_kv.tile([CO, T], f32, tag="kv")
                for mk in range(8):
                    nc.tensor.matmul(kvp, lhsT=d_sb[:, mj * 8 + mk, :],
                                     rhs=p[:, (mk >> 2) & 1, (mk >> 1) & 1, mk & 1, :],
                                     start=(mk == 0), stop=(mk == 7))
                nc.scalar.activation(out=kvs[:, mj, :], in_=kvp, func=ACT.Copy)
                nc.vector.tensor_tensor(h["sqkv"][:, mj, :], kvs[:, mj, :],
                                        kvs[:, mj, :], ALU.mult)

        def phase_C4a(h):
            kvs = h["kvs"]
            sqkv = h["sqkv"]
            t4 = psmall.tile([CO, 4, T], f16, tag="svshared")
            nc.vector.tensor_tensor(t4[:], sqkv[:, 0:4, :], sqkv[:, 4:8, :], ALU.add)
            nc.vector.tensor_tensor(t4[:, 0:2, :], t4[:, 0:2, :], t4[:, 2:4, :], ALU.add)
            n2 = psmall.tile([CO, T], f32, tag="n2")
            nc.vector.tensor_tensor(n2[:], t4[:, 0, :], t4[:, 1, :], ALU.add)
            sroot = psmall.tile([CO, T], f16, tag="sroot")
            nc.scalar.activation(out=sroot, in_=n2, func=ACT.Sqrt)
            csp = psum_kv.tile([1, T], f32, tag="kv")
            nc.tensor.matmul(csp, lhsT=cw_sb, rhs=sroot, start=True, stop=True)
            rb0 = psmall.tile([1, T], f32, tag="rb0")
            nc.vector.tensor_scalar(rb0[:], csp[:], EPS, None, ALU.add)
            rb1 = psmall.tile([1, T], f32, tag="rb1")
            nc.vector.reciprocal_approx_fast(out=rb1, in_=rb0)
            rb1h = psmall.tile([1, T], f16, tag="rb1h")
            nc.vector.tensor_copy(rb1h[:], rb1[:])
            rbp = psum_kv.tile([CO, T], f32, tag="kv")
            nc.tensor.matmul(rbp, lhsT=or_sb, rhs=rb1h, start=True, stop=True)
            rbB = psmall.tile([CO, T], f16, tag="rbB")
            nc.scalar.activation(out=rbB, in_=rbp, func=ACT.Copy)
            h["rbB"] = rbB

        def phase_C4b(h):
            kvs, qt, rbB = h["kvs"], h["qb"], h["rbB"]
            outp = pout.tile([CO, 8, T], f16, tag="outp")
            rbB_b = rbB[:, None, :].to_broadcast((CO, 8, T))
            nc.vector.tensor_tensor(kvs[:], kvs[:], rbB_b, ALU.mult)
            if need_resid:
                nc.vector.tensor_scalar(kvs[:], kvs[:], s_sb[:, 19:20], None, ALU.mult)
            nc.vector.tensor_tensor(outp[:], kvs[:], qt[:], ALU.add)
            h["outp"] = outp

        def phase_D(t, h):
            outp = h["outp"]
            for c in range(nchunk_t):
                r0 = t * T + c * CHUNK
                otp = psum_o.tile([CHUNK, 8, CIN], f16, tag="otp")
                for i in range(8):
                    nc.tensor.transpose(otp[:, i, :],
                                        outp[:, B2M[i], c * CHUNK:(c + 1) * CHUNK],
                                        ident16[:])
                st32 = pout.tile([CHUNK, CIN, 8], f32, tag="st32")
                nc.scalar.activation(out=st32[:].rearrange("p m i -> p i m"),
                                     in_=otp, func=ACT.Copy)
                nc.scalar.dma_start(out=out_ext[r0:r0 + CHUNK], in_=st32)

        for t in range(nt):
            h = phase_AB(t)
            phase_C12(h)
            phase_C3(h, range(8))
            phase_C4a(h)
            phase_C4b(h)
            phase_D(t, h)

    return nc


_NC_CACHE = {}


def _get_nc(n_rows, need_resid):
    key = (n_rows, need_resid)
    if key not in _NC_CACHE:
        _NC_CACHE[key] = _build_nc(n_rows, need_resid)
    return _NC_CACHE[key]


# ---------------------------------------------------------------- entrypoint
def kernel(input, W_qkv, b_qkv, silu_a, silu_b, gp_w, norm_a, ln_a):
    from concourse.bass_utils import run_bass_kernel_spmd

    x = np.ascontiguousarray(np.asarray(input, np.float32))
    consts, need_resid = _host_constants(W_qkv, b_qkv, silu_a, silu_b,
                                         gp_w, norm_a, ln_a)
    nc = _get_nc(BS, need_resid)

    in_maps = []
    for c in range(N_CORES):
        m = {"x": x[c * BS:(c + 1) * BS]}
        m.update(consts)
        in_maps.append(m)

    if not nc.is_finalized():
        nc.finalize()
    res = run_bass_kernel_spmd(nc, in_maps, core_ids=list(range(N_CORES)))
    outs = [res.results[c]["out"].reshape(BS, CIN, 8) for c in range(N_CORES)]
    return np.concatenate(outs, axis=0).astype(np.float32)
